# revision 2
# baseline (speedup 1.0000x reference)
"""Trainium2 Bass kernel for a 2-layer GCN encoder + global mean pool (v5).

Reference computation (PyG GCNConv semantics, eval mode):
    h1 = relu(Ahat @ (x @ W1) + b1)
    h2 = Ahat @ h1 @ W2 + b2
    out = segment_mean(h2, batch)        -> [NUM_GRAPHS, OUT_DIM]
with Ahat = D^-1/2 (A + I) D^-1/2, deg = in-degree + 1.

v5 = v2 (big dma_gather ops, SBUF-resident activations) + hidden layer-2
collective: z1 is exchanged in two half-AllGathers (A = first 49 blocks,
B = rest).  AG2-A is issued as soon as layer 1 finishes block 48 (hides
behind layer 1's second half); AG2-B hides behind layer 2's pass A.  Layer 2
aggregates in two passes (A-windows -> bf16 acc, then reinject + B-windows)
so its gather work only depends on the half-table it consumes.  Layer 1 uses
a single AllGather (nothing available to hide it behind).
"""

import math

import ml_dtypes
import numpy as np

P = 128
N_NODES = 100000
N_EDGES = 1600000
NUM_GRAPHS = 1000
IN_DIM, HID_DIM, OUT_DIM = 256, 128, 64
N_CORES = 8

BF16 = ml_dtypes.bfloat16
PAD_SLOT = 255.0  # one-hot build never matches iota 0..127

GRP = 7          # dst blocks per gather group
N_Q = 4          # src windows (int16 index range)


class Plan:
    pass


class Tables:
    pass


def _edge_tables(core, g, slot, q, lid, n_cores, n_blk, n_bg):
    """Bucket edges by (dst block g, src window q); pad each (g, q) segment
    to whole 128-edge chunks (SPMD-uniform max over cores).  Returns gather
    (op-major) and slots (block-major) layouts."""
    t = Tables()
    key = (core * n_blk + g) * N_Q + q
    counts = np.bincount(key, minlength=n_cores * n_blk * N_Q)
    counts = counts.reshape(n_cores, n_blk, N_Q)
    m = np.ceil(counts.max(axis=0) / P).astype(np.int64)  # [n_blk, N_Q]
    t.m = m

    C_op = np.zeros((n_bg, N_Q), dtype=np.int64)
    opoff = np.zeros((n_blk, N_Q), dtype=np.int64)
    for bg in range(n_bg):
        lo, hi = bg * GRP, min((bg + 1) * GRP, n_blk)
        for qq in range(N_Q):
            c = 0
            for gg in range(lo, hi):
                opoff[gg, qq] = c
                c += int(m[gg, qq])
            C_op[bg, qq] = c
    gtile_base = np.concatenate(
        [np.zeros((n_bg, 1), np.int64), np.cumsum(C_op, axis=1)], axis=1)
    ixcol_base = np.concatenate([[0], np.cumsum(8 * C_op.reshape(-1))])
    t.C_op, t.gtile_base, t.opoff = C_op, gtile_base, opoff
    t.ixcol_base = ixcol_base
    t.n_ixcol = int(ixcol_base[-1])

    M_g = m.sum(axis=1)
    sbase = np.concatenate([[0], np.cumsum(M_g)])
    moff = np.concatenate(
        [np.zeros((n_blk, 1), np.int64), np.cumsum(m, axis=1)], axis=1)
    t.M_g, t.sbase, t.moff = M_g, sbase, moff
    t.C_tot = int(sbase[-1])

    order = np.argsort(key, kind="stable")
    seg_start = np.concatenate([[0], np.cumsum(counts.reshape(-1))])[:-1]
    rank = np.arange(len(order)) - seg_start[key[order]]
    core_o, g_o, q_o = core[order], g[order], q[order]
    ck = rank // P
    pp = (rank % P).astype(np.int64)
    opid = (g_o // GRP) * N_Q + q_o
    j = (opoff[g_o, q_o] + ck) * P + pp
    ixcol = ixcol_base[opid] + j // 16
    idx_all = np.zeros((n_cores, 16, t.n_ixcol), dtype=np.int16)
    idx_all[core_o, j % 16, ixcol] = lid[order]
    t.idx_all = np.tile(idx_all, (1, 8, 1))

    scol = sbase[g_o] + moff[g_o, q_o] + ck
    slots_all = np.full((n_cores, P, t.C_tot), PAD_SLOT, dtype=BF16)
    slots_all[core_o, pp, scol] = slot[order].astype(BF16)
    t.slots_all = slots_all
    return t


def make_plan(x, W1, b1, W2, b2, edge_index, batch,
              n_nodes=N_NODES, num_graphs=NUM_GRAPHS, n_cores=N_CORES):
    pl = Plan()
    n_pc = n_nodes // n_cores
    assert n_pc * n_cores == n_nodes
    n_blk = math.ceil(n_pc / P)
    n_pad = n_blk * P
    pl.n_nodes, pl.num_graphs, pl.n_cores = n_nodes, num_graphs, n_cores
    pl.n_pc, pl.n_blk, pl.n_pad = n_pc, n_blk, n_pad
    pl.d_in, pl.d_hid, pl.d_out = x.shape[1], W1.shape[1], W2.shape[1]
    d_in, d_hid, d_out = pl.d_in, pl.d_hid, pl.d_out

    n_bg = math.ceil(n_blk / GRP)
    pl.n_bg = n_bg
    q_rows = n_nodes // N_Q                    # layer-1 window (25000)
    assert q_rows * N_Q == n_nodes and q_rows < 32768
    pl.q_rows = q_rows
    blkA = (n_blk + 1) // 2                    # A half: blocks [0, blkA)
    rA = blkA * P
    rB = n_pc - rA
    wA = (n_cores * rA) // 2                   # layer-2 A window (25088)
    wB = (n_cores * rB + 1) // 2               # layer-2 B window (24912)
    assert wA < 32768 and wB < 32768
    pl.blkA, pl.rA, pl.rB, pl.wA, pl.wB = blkA, rA, rB, wA, wB

    src = np.asarray(edge_index[0], dtype=np.int64)
    dst = np.asarray(edge_index[1], dtype=np.int64)
    batch = np.asarray(batch, dtype=np.int64)

    deg = np.bincount(dst, minlength=n_nodes).astype(np.float64) + 1.0
    dis = (1.0 / np.sqrt(deg)).astype(np.float32)

    core = dst // n_pc
    loc = dst - core * n_pc
    g = loc // P
    slot = loc % P

    # layer-1 windows: identity node id, 4 consecutive ranges
    q1 = src // q_rows
    lid1 = (src - q1 * q_rows).astype(np.int16)
    pl.t1 = _edge_tables(core, g, slot, q1, lid1, n_cores, n_blk, n_bg)

    # layer-2 windows: A/B half-shard spaces (2 windows each)
    core_s = src // n_pc
    loc_s = src - core_s * n_pc
    half = (loc_s >= rA).astype(np.int64)
    hrid = np.where(half == 0, core_s * rA + loc_s,
                    core_s * rB + (loc_s - rA))
    wsz = np.where(half == 0, wA, wB)
    q2 = 2 * half + hrid // wsz
    lid2 = (hrid % wsz).astype(np.int16)
    pl.t2 = _edge_tables(core, g, slot, q2, lid2, n_cores, n_blk, n_bg)

    # per-node scalars laid out [core][P, n_blk] (partition p, block g)
    def node_layout(vals, pad=0.0):
        out = np.full((n_cores, P, n_blk), pad, dtype=np.float32)
        v = vals.reshape(n_cores, n_pc)
        for k in range(n_cores):
            full = np.full(n_pad, pad, dtype=np.float32)
            full[:n_pc] = v[k]
            out[k] = full.reshape(n_blk, P).T
        return out

    pl.dis_t = node_layout(dis)
    cnt = np.bincount(batch, minlength=num_graphs).astype(np.float64)
    recip_g = (1.0 / np.maximum(cnt, 1.0)).astype(np.float32)
    pl.recip_t = node_layout(recip_g[batch])

    # pooling: groups of G_CH node-chunks share a PSUM tile
    G_CH = 13
    while True:
        n_grp = math.ceil(n_blk / G_CH)
        ok = True
        pool_slots = np.full((n_cores, P, n_blk), PAD_SLOT, dtype=np.float32)
        pool_base = np.zeros((n_cores, n_grp), dtype=np.int64)
        for k in range(n_cores):
            b = batch[k * n_pc:(k + 1) * n_pc]
            for gg in range(n_grp):
                lo = gg * G_CH * P
                if lo >= n_pc:
                    pool_base[k, gg] = 0
                    continue
                hi = min((gg + 1) * G_CH * P, n_pc)
                base = b[lo]
                pool_base[k, gg] = base
                rel = b[lo:hi] - base
                if rel.max() >= P:
                    ok = False
                    break
                sl = np.full(min((gg + 1) * G_CH * P, n_blk * P) - lo, PAD_SLOT,
                             dtype=np.float32)
                sl[:hi - lo] = rel
                dstv = pool_slots[k].T.reshape(-1)
                dstv[lo:lo + len(sl)] = sl
                pool_slots[k] = dstv.reshape(n_blk, P).T
            if not ok:
                break
        if ok:
            break
        G_CH //= 2
        assert G_CH >= 1
    pl.G_CH, pl.n_grp = G_CH, n_grp
    pl.pool_slots, pl.pool_base = pool_slots, pool_base

    # x shards pre-transposed: xT[core][f, node] (padded nodes), bf16
    x = np.asarray(x, dtype=np.float32)
    xT = np.zeros((n_cores, d_in, n_pad), dtype=BF16)
    xs = x.reshape(n_cores, n_pc, d_in).astype(BF16)
    for k in range(n_cores):
        xT[k, :, :n_pc] = xs[k].T
    pl.xT = xT

    W1 = np.asarray(W1, dtype=np.float32)
    W2 = np.asarray(W2, dtype=np.float32)
    kk = d_in // P
    pl.w1t = np.concatenate([W1[k * P:(k + 1) * P] for k in range(kk)],
                            axis=1).astype(BF16)
    pl.n_k1 = kk
    pl.w2_sb = W2.astype(BF16)

    pl.b1b = np.broadcast_to(np.asarray(b1, np.float32), (P, d_hid)).copy()
    pl.b2b = np.broadcast_to(np.asarray(b2, np.float32), (P, d_out)).copy()
    iot = np.broadcast_to(np.arange(P, dtype=np.float32), (P, P))
    pl.iotab = iot.astype(BF16).copy()
    pl.iotaf = iot.astype(np.float32).copy()
    pl.ident = np.eye(P, dtype=BF16)
    return pl


def build_program(pl, body_repeat=1, ablate=()):
    import concourse.bass as bass
    import concourse.mybir as mybir
    import concourse.tile as tile
    from concourse import bacc

    f32 = mybir.dt.float32
    bf16 = mybir.dt.bfloat16
    i16 = mybir.dt.int16
    AF = mybir.ActivationFunctionType
    OP = mybir.AluOpType

    n_pc, n_blk, n_pad = pl.n_pc, pl.n_blk, pl.n_pad
    d_in, d_hid, d_out = pl.d_in, pl.d_hid, pl.d_out
    n_cores, n_bg = pl.n_cores, pl.n_bg
    q_rows = pl.q_rows
    blkA, rA, rB, wA, wB = pl.blkA, pl.rA, pl.rB, pl.wA, pl.wB
    t1, t2 = pl.t1, pl.t2

    nc = bacc.Bacc("TRN2", target_bir_lowering=False, debug=False,
                   num_devices=n_cores)

    # --- I/O ---
    xT_d = nc.dram_tensor("xT", [d_in, n_pad], bf16, kind="ExternalInput")
    w1t_d = nc.dram_tensor("w1t", [P, pl.n_k1 * d_hid], bf16, kind="ExternalInput")
    w2_d = nc.dram_tensor("w2", [d_hid, d_out], bf16, kind="ExternalInput")
    b1b_d = nc.dram_tensor("b1b", [P, d_hid], f32, kind="ExternalInput")
    b2b_d = nc.dram_tensor("b2b", [P, d_out], f32, kind="ExternalInput")
    iotab_d = nc.dram_tensor("iotab", [P, P], bf16, kind="ExternalInput")
    iotaf_d = nc.dram_tensor("iotaf", [P, P], f32, kind="ExternalInput")
    dis_d = nc.dram_tensor("dis_t", [P, n_blk], f32, kind="ExternalInput")
    recip_d = nc.dram_tensor("recip_t", [P, n_blk], f32, kind="ExternalInput")
    idx1_d = nc.dram_tensor("idx1", [P, t1.n_ixcol], i16, kind="ExternalInput")
    slots1_d = nc.dram_tensor("slots1", [P, t1.C_tot], bf16, kind="ExternalInput")
    idx2_d = nc.dram_tensor("idx2", [P, t2.n_ixcol], i16, kind="ExternalInput")
    slots2_d = nc.dram_tensor("slots2", [P, t2.C_tot], bf16, kind="ExternalInput")
    pslots_d = nc.dram_tensor("pool_slots", [P, n_blk], f32, kind="ExternalInput")
    ident_d = nc.dram_tensor("ident", [P, P], bf16, kind="ExternalInput")

    pool_part = nc.dram_tensor("pool_part", [pl.n_grp * P, d_out], f32,
                               kind="ExternalOutput")

    # --- internal DRAM ---
    y1_sh = nc.dram_tensor("y1_sh", [n_pad, d_hid], bf16)
    z1_shA = nc.dram_tensor("z1_shA", [rA, d_hid], bf16)
    z1_shB = nc.dram_tensor("z1_shB", [n_pad - rA, d_hid], bf16)
    y1_full = nc.dram_tensor("y1_full", [pl.n_nodes, d_hid], bf16,
                             addr_space="Shared")
    y2fullA = nc.dram_tensor("y2fullA", [n_cores * rA, d_hid], bf16,
                             addr_space="Shared")
    y2fullB = nc.dram_tensor("y2fullB", [n_cores * rB, d_hid], bf16,
                             addr_space="Shared")

    groups = [list(range(n_cores))]

    with tile.TileContext(nc) as tc:
        with (
            tc.tile_pool(name="const", bufs=1) as cpool,
            tc.tile_pool(name="resid", bufs=1) as rpool,
            tc.tile_pool(name="xin", bufs=2) as xpool,
            tc.tile_pool(name="gath", bufs=2) as gpool,
            tc.tile_pool(name="ixp", bufs=2) as ixpool,
            tc.tile_pool(name="onehot", bufs=3) as mpool,
            tc.tile_pool(name="eplg", bufs=3) as epool,
            tc.tile_pool(name="ps_fe", bufs=2, space="PSUM") as ps_fe,
            tc.tile_pool(name="ps_agg", bufs=2, space="PSUM") as ps_agg,
            tc.tile_pool(name="ps_o", bufs=2, space="PSUM") as ps_o,
            tc.tile_pool(name="ps_p", bufs=2, space="PSUM") as ps_p,
        ):
            w1_sb = cpool.tile([P, pl.n_k1 * d_hid], bf16)
            nc.sync.dma_start(out=w1_sb[:], in_=w1t_d[:, :])
            w2_sb = cpool.tile([d_hid, d_out], bf16)
            nc.sync.dma_start(out=w2_sb[:], in_=w2_d[:, :])
            b1_sb = cpool.tile([P, d_hid], f32)
            nc.sync.dma_start(out=b1_sb[:], in_=b1b_d[:, :])
            b2_sb = cpool.tile([P, d_out], f32)
            nc.sync.dma_start(out=b2_sb[:], in_=b2b_d[:, :])
            iob_sb = cpool.tile([P, P], bf16)
            nc.sync.dma_start(out=iob_sb[:], in_=iotab_d[:, :])
            iof_sb = cpool.tile([P, P], f32)
            nc.sync.dma_start(out=iof_sb[:], in_=iotaf_d[:, :])
            dis_sb = cpool.tile([P, n_blk], f32)
            nc.sync.dma_start(out=dis_sb[:], in_=dis_d[:, :])
            recip_sb = cpool.tile([P, n_blk], f32)
            nc.sync.dma_start(out=recip_sb[:], in_=recip_d[:, :])
            slots1_sb = cpool.tile([P, t1.C_tot], bf16)
            nc.sync.dma_start(out=slots1_sb[:], in_=slots1_d[:, :])
            slots2_sb = cpool.tile([P, t2.C_tot], bf16)
            nc.sync.dma_start(out=slots2_sb[:], in_=slots2_d[:, :])
            pslots_sb = cpool.tile([P, n_blk], f32)
            nc.sync.dma_start(out=pslots_sb[:], in_=pslots_d[:, :])
            ident_sb = cpool.tile([P, P], bf16)
            nc.sync.dma_start(out=ident_sb[:], in_=ident_d[:, :])

            for _rep in range(body_repeat):
                y1_res = rpool.tile([P, n_blk * d_hid], bf16, tag="y1res")
                z1_res = rpool.tile([P, n_blk * d_hid], bf16, tag="z1res")
                z2_res = rpool.tile([P, n_blk * d_out], f32, tag="z2res")

                def emit_coll(ins_ap, outs_ap):
                    if "coll" not in ablate:
                        nc.gpsimd.collective_compute(
                            "AllGather", OP.bypass, replica_groups=groups,
                            ins=[ins_ap], outs=[outs_ap])

                def z_sh_dst(gg):
                    if gg < blkA:
                        return z1_shA[gg * P:(gg + 1) * P, :]
                    return z1_shB[(gg - blkA) * P:(gg - blkA + 1) * P, :]

                # ---------- front-end: y1 = dis * (x @ W1) ----------
                for bg in range(n_bg):
                    lo = bg * GRP
                    hi = min((bg + 1) * GRP, n_blk)
                    w = (hi - lo) * P
                    xk = []
                    for k in range(pl.n_k1):
                        xt = xpool.tile([P, w], bf16, tag=f"xT{k}")
                        nc.sync.dma_start(
                            out=xt[:],
                            in_=xT_d[k * P:(k + 1) * P, lo * P:lo * P + w])
                        xk.append(xt)
                    for gg in range(lo, hi):
                        c0 = (gg - lo) * P
                        psum_h = ps_fe.tile([P, d_hid], f32, tag="feps")
                        for k in range(pl.n_k1):
                            nc.tensor.matmul(
                                psum_h[:], lhsT=xk[k][:, c0:c0 + P],
                                rhs=w1_sb[:, k * d_hid:(k + 1) * d_hid],
                                start=(k == 0), stop=(k == pl.n_k1 - 1))
                        nc.scalar.activation(
                            y1_res[:, gg * d_hid:(gg + 1) * d_hid],
                            psum_h[:], AF.Copy, scale=dis_sb[:, gg:gg + 1])
                        nc.sync.dma_start(
                            out=y1_sh[gg * P:(gg + 1) * P, :],
                            in_=y1_res[:, gg * d_hid:(gg + 1) * d_hid])

                emit_coll(y1_sh[0:n_pc, :], y1_full[:, :])

                # ---------- layer 1 (single pass over 4 windows) ----------
                for bg in range(n_bg):
                    lo = bg * GRP
                    hi = min((bg + 1) * GRP, n_blk)
                    cbg = int(t1.gtile_base[bg, N_Q])
                    ix = ixpool.tile([P, 8 * cbg], i16, tag="ix")
                    nc.sync.dma_start(
                        out=ix[:],
                        in_=idx1_d[:, t1.ixcol_base[bg * N_Q]:
                                   t1.ixcol_base[bg * N_Q] + 8 * cbg])
                    gt = gpool.tile([P, cbg * d_hid], bf16, tag="gath")
                    for qq in range(N_Q):
                        cop = int(t1.C_op[bg, qq])
                        if cop == 0:
                            continue
                        a = int(t1.gtile_base[bg, qq])
                        if "gather" not in ablate:
                            nc.gpsimd.dma_gather(
                                gt[:, a * d_hid:(a + cop) * d_hid].rearrange(
                                    "p (c e) -> p c e", e=d_hid),
                                y1_full[qq * q_rows:(qq + 1) * q_rows, :],
                                ix[:, 8 * a:8 * (a + cop)],
                                cop * P, cop * P, d_hid, single_packet=False)
                    for gg in range(lo, hi):
                        mg = int(t1.M_g[gg])
                        sb0 = int(t1.sbase[gg])
                        mt = mpool.tile([P, max(mg, 1) * P], bf16, tag="oh")
                        if "onehot" not in ablate and mg > 0:
                            nc.vector.tensor_tensor(
                                out=mt[:, 0:mg * P].rearrange(
                                    "p (c q) -> p c q", q=P),
                                in0=slots1_sb[:, sb0:sb0 + mg].to_broadcast(
                                    [P, mg, P]),
                                in1=iob_sb[:, None, :].to_broadcast([P, mg, P]),
                                op=OP.is_equal)
                        nomm = mg == 0 or "mm" in ablate
                        psum_a = ps_agg.tile([P, d_hid], f32, tag="agg")
                        nc.tensor.matmul(
                            psum_a[:], lhsT=ident_sb[:],
                            rhs=y1_res[:, gg * d_hid:(gg + 1) * d_hid],
                            start=True, stop=nomm)
                        done = 0
                        for qq in range(N_Q):
                            mq = int(t1.m[gg, qq])
                            gc0 = int(t1.gtile_base[bg, qq] + t1.opoff[gg, qq])
                            mc0 = int(t1.moff[gg, qq])
                            for k in range(mq):
                                done += 1
                                if "mm" in ablate:
                                    continue
                                nc.tensor.matmul(
                                    psum_a[:],
                                    lhsT=mt[:, (mc0 + k) * P:(mc0 + k + 1) * P],
                                    rhs=gt[:, (gc0 + k) * d_hid:
                                           (gc0 + k + 1) * d_hid],
                                    start=False, stop=(done == mg))
                        t1e = epool.tile([P, d_hid], f32, tag="ep1")
                        nc.scalar.activation(t1e[:], psum_a[:], AF.Copy,
                                             scale=dis_sb[:, gg:gg + 1])
                        t2e = epool.tile([P, d_hid], f32, tag="ep2")
                        nc.vector.tensor_tensor(t2e[:], t1e[:], b1_sb[:],
                                                op=OP.add)
                        nc.vector.tensor_scalar(
                            out=z1_res[:, gg * d_hid:(gg + 1) * d_hid],
                            in0=t2e[:], scalar1=0.0,
                            scalar2=dis_sb[:, gg:gg + 1],
                            op0=OP.max, op1=OP.mult)
                        nc.sync.dma_start(
                            out=z_sh_dst(gg),
                            in_=z1_res[:, gg * d_hid:(gg + 1) * d_hid])
                        if gg == blkA - 1:
                            emit_coll(z1_shA[0:rA, :], y2fullA[:, :])
                emit_coll(z1_shB[0:rB, :], y2fullB[:, :])

                # ---------- layer 2 (two passes over A/B half tables) ------
                def win2_ap(qq):
                    if qq == 0:
                        return y2fullA[0:wA, :]
                    if qq == 1:
                        return y2fullA[wA:2 * wA, :]
                    if qq == 2:
                        return y2fullB[0:wB, :]
                    return y2fullB[wB:2 * wB, :]

                for half in (0, 1):
                    qs = (0, 1) if half == 0 else (2, 3)
                    for bg in range(n_bg):
                        lo = bg * GRP
                        hi = min((bg + 1) * GRP, n_blk)
                        a0 = int(t2.gtile_base[bg, qs[0]])
                        a1 = int(t2.gtile_base[bg, qs[1] + 1])
                        chalf = a1 - a0
                        ix = ixpool.tile([P, 8 * max(chalf, 1)], i16,
                                         tag="ix")
                        if chalf > 0:
                            nc.sync.dma_start(
                                out=ix[:, 0:8 * chalf],
                                in_=idx2_d[:, t2.ixcol_base[bg * N_Q + qs[0]]:
                                           t2.ixcol_base[bg * N_Q + qs[0]]
                                           + 8 * chalf])
                        gt = gpool.tile([P, max(chalf, 1) * d_hid], bf16,
                                        tag="gath")
                        for qq in qs:
                            cop = int(t2.C_op[bg, qq])
                            if cop == 0:
                                continue
                            a = int(t2.gtile_base[bg, qq]) - a0
                            if "gather" not in ablate:
                                nc.gpsimd.dma_gather(
                                    gt[:, a * d_hid:(a + cop) * d_hid].rearrange(
                                        "p (c e) -> p c e", e=d_hid),
                                    win2_ap(qq),
                                    ix[:, 8 * a:8 * (a + cop)],
                                    cop * P, cop * P, d_hid, single_packet=False)
                        for gg in range(lo, hi):
                            sb0 = int(t2.sbase[gg]) + int(t2.moff[gg, qs[0]])
                            mh = int(t2.moff[gg, qs[1] + 1]
                                     - t2.moff[gg, qs[0]])
                            mt = mpool.tile([P, max(mh, 1) * P], bf16,
                                            tag="oh")
                            if "onehot" not in ablate and mh > 0:
                                nc.vector.tensor_tensor(
                                    out=mt[:, 0:mh * P].rearrange(
                                        "p (c q) -> p c q", q=P),
                                    in0=slots2_sb[:, sb0:sb0 + mh].to_broadcast(
                                        [P, mh, P]),
                                    in1=iob_sb[:, None, :].to_broadcast(
                                        [P, mh, P]),
                                    op=OP.is_equal)
                            acc_sl = z1_res[:, gg * d_hid:(gg + 1) * d_hid]
                            nomm = mh == 0 or "mm" in ablate
                            psum_a = ps_agg.tile([P, P], f32, tag="agg")
                            if half == 0:
                                nc.tensor.matmul(
                                    psum_a[:], lhsT=acc_sl, rhs=ident_sb[:],
                                    start=True, stop=nomm)
                            else:
                                nc.tensor.matmul(
                                    psum_a[:], lhsT=ident_sb[:], rhs=acc_sl,
                                    start=True, stop=nomm)
                            done = 0
                            for qq in qs:
                                mq = int(t2.m[gg, qq])
                                gc0 = int(t2.gtile_base[bg, qq] - a0
                                          + t2.opoff[gg, qq])
                                mc0 = int(t2.moff[gg, qq] - t2.moff[gg, qs[0]])
                                for k in range(mq):
                                    done += 1
                                    if "mm" in ablate:
                                        continue
                                    nc.tensor.matmul(
                                        psum_a[:],
                                        lhsT=gt[:, (gc0 + k) * d_hid:
                                                (gc0 + k + 1) * d_hid],
                                        rhs=mt[:, (mc0 + k) * P:
                                               (mc0 + k + 1) * P],
                                        start=False, stop=(done == mh))
                            if half == 0:
                                nc.scalar.activation(acc_sl, psum_a[:],
                                                     AF.Copy)
                                continue
                            s2t = epool.tile([P, P], bf16, tag="s2t")
                            nc.scalar.activation(s2t[:], psum_a[:], AF.Copy)
                            psum_o = ps_o.tile([P, d_out], f32, tag="out2")
                            nc.tensor.matmul(psum_o[:], lhsT=s2t[:],
                                             rhs=w2_sb[:], start=True,
                                             stop=True)
                            t3e = epool.tile([P, d_out], f32, tag="ep3")
                            nc.scalar.activation(t3e[:], psum_o[:], AF.Copy,
                                                 scale=dis_sb[:, gg:gg + 1])
                            t4e = epool.tile([P, d_out], f32, tag="ep4")
                            nc.vector.tensor_tensor(t4e[:], t3e[:], b2_sb[:],
                                                    op=OP.add)
                            nc.vector.tensor_scalar(
                                out=z2_res[:, gg * d_out:(gg + 1) * d_out],
                                in0=t4e[:], scalar1=recip_sb[:, gg:gg + 1],
                                scalar2=None, op0=OP.mult)

                # ---------- pool ----------
                for grp in range(pl.n_grp):
                    lo = grp * pl.G_CH
                    hi = min((grp + 1) * pl.G_CH, n_blk)
                    psum_p = ps_p.tile([P, d_out], f32, tag="pool")
                    for jj, cblk in enumerate(range(lo, hi)):
                        mp = epool.tile([P, P], f32, tag="poolM")
                        nc.vector.tensor_tensor(
                            out=mp[:],
                            in0=pslots_sb[:, cblk:cblk + 1].to_broadcast([P, P]),
                            in1=iof_sb[:], op=OP.is_equal)
                        nc.tensor.matmul(
                            psum_p[:], lhsT=mp[:],
                            rhs=z2_res[:, cblk * d_out:(cblk + 1) * d_out],
                            start=(jj == 0), stop=(jj == hi - lo - 1))
                    pout = epool.tile([P, d_out], f32, tag="pout")
                    nc.vector.tensor_copy(out=pout[:], in_=psum_p[:])
                    nc.sync.dma_start(out=pool_part[grp * P:(grp + 1) * P, :],
                                      in_=pout[:])

    nc.compile()
    return nc


def make_in_maps(pl):
    maps = []
    for k in range(pl.n_cores):
        maps.append({
            "xT": pl.xT[k],
            "w1t": pl.w1t,
            "w2": pl.w2_sb,
            "b1b": pl.b1b,
            "b2b": pl.b2b,
            "iotab": pl.iotab,
            "iotaf": pl.iotaf,
            "dis_t": pl.dis_t[k],
            "recip_t": pl.recip_t[k],
            "idx1": pl.t1.idx_all[k],
            "slots1": pl.t1.slots_all[k],
            "idx2": pl.t2.idx_all[k],
            "slots2": pl.t2.slots_all[k],
            "pool_slots": pl.pool_slots[k],
            "ident": pl.ident,
        })
    return maps


def combine_outputs(pl, parts):
    out = np.zeros((pl.num_graphs, pl.d_out), dtype=np.float32)
    for k in range(pl.n_cores):
        pp = np.asarray(parts[k], dtype=np.float32).reshape(pl.n_grp, P, pl.d_out)
        for g in range(pl.n_grp):
            base = int(pl.pool_base[k, g])
            n = min(P, pl.num_graphs - base)
            if n > 0:
                out[base:base + n] += pp[g, :n]
    return out


def prepare(x, W1, b1, W2, b2, edge_index, batch):
    pl = make_plan(x, W1, b1, W2, b2, edge_index, batch)
    nc = build_program(pl)
    in_maps = make_in_maps(pl)
    return pl, nc, in_maps


def kernel(x, W1, b1, W2, b2, edge_index, batch):
    from concourse.bass_utils import run_bass_kernel_spmd

    pl, nc, in_maps = prepare(x, W1, b1, W2, b2, edge_index, batch)
    res = run_bass_kernel_spmd(nc, in_maps, list(range(pl.n_cores)))
    parts = [res.results[k]["pool_part"] for k in range(pl.n_cores)]
    return combine_outputs(pl, parts)


def make_pjrt_runner(nc, in_maps, n_cores):
    """Build a jitted 8-core runner (mirrors bass2jax.run_bass_via_pjrt, but
    without donation so the executable can be re-invoked for timing)."""
    import jax
    import numpy as np
    from jax.sharding import Mesh, PartitionSpec
    from jax.experimental.shard_map import shard_map
    import concourse.mybir as mybir
    from concourse.bass2jax import (
        _bass_exec_p, install_neuronx_cc_hook, partition_id_tensor)

    install_neuronx_cc_hook()
    assert nc.dbg_addr is None or not nc.dbg_callbacks

    partition_name = nc.partition_id_tensor.name if nc.partition_id_tensor else None
    in_names, out_names, out_avals, zero_outs = [], [], [], []
    for alloc in nc.m.functions[0].allocations:
        if not isinstance(alloc, mybir.MemoryLocationSet):
            continue
        name = alloc.memorylocations[0].name
        if alloc.kind == "ExternalInput":
            if name != partition_name:
                in_names.append(name)
        elif alloc.kind == "ExternalOutput":
            shape = tuple(alloc.tensor_shape)
            dtype = mybir.dt.np(alloc.dtype)
            out_names.append(name)
            out_avals.append(jax.core.ShapedArray(shape, dtype))
            zero_outs.append(np.zeros(shape, dtype))
    n_params = len(in_names)
    all_names = list(in_names) + list(out_names)
    if partition_name is not None:
        all_names.append(partition_name)

    def _body(*args):
        operands = list(args)
        if partition_name is not None:
            operands.append(partition_id_tensor())
        outs = _bass_exec_p.bind(
            *operands,
            out_avals=tuple(out_avals),
            in_names=tuple(all_names),
            out_names=tuple(out_names),
            lowering_input_output_aliases=(),
            sim_require_finite=True,
            sim_require_nnan=True,
            nc=nc,
        )
        return tuple(outs)

    devices = jax.devices()[:n_cores]
    mesh = Mesh(np.asarray(devices), ("core",))
    n_outs = len(out_names)
    in_specs = (PartitionSpec("core"),) * (n_params + n_outs)
    out_specs = (PartitionSpec("core"),) * n_outs
    fn = jax.jit(shard_map(_body, mesh=mesh, in_specs=in_specs,
                           out_specs=out_specs, check_rep=False),
                 keep_unused=True)
    per_core = [[np.asarray(m[name]) for name in in_names] for m in in_maps]
    concat_in = [np.concatenate([per_core[c][i] for c in range(n_cores)], axis=0)
                 for i in range(n_params)]
    concat_zeros = [np.zeros((n_cores * z.shape[0], *z.shape[1:]), z.dtype)
                    for z in zero_outs]
    args = concat_in + concat_zeros
    out_shapes = [a.shape for a in out_avals]
    return fn, args, out_names, out_shapes
